# revision 1
# baseline (speedup 1.0000x reference)
"""Trainium2 Bass kernel for nn_HadaMard: fused proj + 2xLayerNorm + outer product.

Reference computation (per batch b):
  qf = q[b].reshape(C1, N)           # [1024, 1024]  (C1 on rows, N=H*W cols)
  proj = Wp @ qf + bp                # [256, 1024]
  qn = LN_over_d(proj) * g1 + b1     # LN over the 256-channel dim
  xn = LN_over_e(x[b]) * g2 + b2     # LN over the 32-channel dim
  out[d*32+e, n] = qn[d, n] * xn[e, n]   # [8192, 1024]

Sharding: data-parallel over B=8, one batch per NeuronCore.

On-chip layout is [channel, n] everywhere (zero transposes):
  - proj = WpT.T @ q via PE (WpT host-transposed, q natural layout)
  - LN stats over the partition axis via ones-matmuls: lhsT = ones*(1/C)
    gives the mean broadcast to all 128 partitions for free.
  - outer product: stationary S4 [4,128] (S4[j,p] = 1 if p//32 == j) broadcasts
    4 qn rows -> 128 partitions in PSUM; one DVE tensor_mul against a
    replicated xn tile -> output tile [128, 1024] -> contiguous 512KB DMA.
"""

import numpy as np

_CACHE = {}

B, C1, H, W = 8, 1024, 32, 32
C2 = 32
Cp = 256
N = H * W  # 1024
CD = Cp * C2  # 8192
EPS = 1e-5


def _build_nc(trace_label=False):
    import os

    import concourse.bacc as bacc
    import concourse.bass as bass
    import concourse.mybir as mybir
    import concourse.tile as tile

    f32r_proj = os.environ.get("HM_F32R_PROJ", "0") == "1"
    f32r_stats = os.environ.get("HM_F32R_STATS", "0") == "1"
    f32r_sel = os.environ.get("HM_F32R_SEL", "0") == "1"
    simple = os.environ.get("HM_SIMPLE", "0") == "1"  # g1=1,b1=0,g2=1,b2=0,bp=0
    split = os.environ.get("HM_SPLIT", "1") == "1"  # bf16 hi/lo selection matmuls

    F32 = mybir.dt.float32
    F32R = mybir.dt.float32r
    BF16 = mybir.dt.bfloat16
    MULT = mybir.AluOpType.mult
    ADD = mybir.AluOpType.add
    SQRT = mybir.ActivationFunctionType.Sqrt

    nc = bacc.Bacc(None, target_bir_lowering=False)

    qh_d = nc.dram_tensor("qh", [C1, N], BF16, kind="ExternalInput")
    ql_d = nc.dram_tensor("ql", [C1, N], BF16, kind="ExternalInput")
    x_d = nc.dram_tensor("x", [C2, N], F32, kind="ExternalInput")
    wh_d = nc.dram_tensor("wh", [C1, Cp], BF16, kind="ExternalInput")
    wl_d = nc.dram_tensor("wl", [C1, Cp], BF16, kind="ExternalInput")
    bp_d = nc.dram_tensor("bpc", [128, 2], F32, kind="ExternalInput")
    g1_d = nc.dram_tensor("g1c", [128, 2], F32, kind="ExternalInput")
    b1_d = nc.dram_tensor("b1c", [128, 2], F32, kind="ExternalInput")
    g2_d = nc.dram_tensor("g2r", [128, 1], F32, kind="ExternalInput")
    b2_d = nc.dram_tensor("b2r", [128, 1], F32, kind="ExternalInput")
    rep_d = nc.dram_tensor(
        "rep", [128, 16 * 128], BF16 if split else F32, kind="ExternalInput"
    )
    sx_d = nc.dram_tensor("sx", [C2, 128], F32, kind="ExternalInput")
    out_d = nc.dram_tensor("out", [CD, N], F32, kind="ExternalOutput")

    with tile.TileContext(nc) as tc:
        with (
            tc.tile_pool(name="cst", bufs=1) as cst,
            tc.tile_pool(name="big", bufs=1) as big,
            tc.tile_pool(name="wrk", bufs=2) as wrk,
            tc.tile_pool(name="stt", bufs=1) as stt,
            tc.tile_pool(name="ost", bufs=4) as ost,
            tc.tile_pool(name="ps", bufs=4, space=bass.MemorySpace.PSUM) as ps,
        ):
            # ---- input loads ----
            qh_sb, ql_sb, wh_sb, wl_sb = [], [], [], []
            for k in range(8):
                t = big.tile([128, N], BF16, tag=f"qh{k}")
                nc.sync.dma_start(t[:], qh_d[128 * k : 128 * (k + 1), :])
                qh_sb.append(t)
                t = big.tile([128, N], BF16, tag=f"ql{k}")
                nc.scalar.dma_start(t[:], ql_d[128 * k : 128 * (k + 1), :])
                ql_sb.append(t)
                t = big.tile([128, Cp], BF16, tag=f"wh{k}")
                nc.sync.dma_start(t[:], wh_d[128 * k : 128 * (k + 1), :])
                wh_sb.append(t)
                t = big.tile([128, Cp], BF16, tag=f"wl{k}")
                nc.scalar.dma_start(t[:], wl_d[128 * k : 128 * (k + 1), :])
                wl_sb.append(t)
            x_sb = cst.tile([C2, N], F32, tag="xs")
            nc.sync.dma_start(x_sb[:], x_d[:])

            def cload(dram, shape, tag):
                t = cst.tile(shape, F32, tag=tag)
                nc.sync.dma_start(t[:], dram[:])
                return t

            bp_sb = cload(bp_d, [128, 2], "bp")
            g1_sb = cload(g1_d, [128, 2], "g1")
            b1_sb = cload(b1_d, [128, 2], "b1")
            g2_sb = cload(g2_d, [128, 1], "g2")
            b2_sb = cload(b2_d, [128, 1], "b2")
            cq_sb = cst.tile([128, 128], F32, tag="cq")
            nc.vector.memset(cq_sb[:], 1.0 / Cp)
            cx_sb = cst.tile([C2, 128], F32, tag="cx")
            nc.vector.memset(cx_sb[:], 1.0 / C2)
            rep_sb = cst.tile([128, 16 * 128], BF16 if split else F32, tag="rep")
            nc.sync.dma_start(rep_sb[:], rep_d[:])
            sx_sb = cload(sx_d, [C2, 128], "sx")
            eps_t = cst.tile([128, 1], F32, tag="eps")
            nc.vector.memset(eps_t[:], EPS)

            def mm_dt(ap, on):
                return ap.bitcast(F32R) if on else ap

            # ---- projection: proj[d, n] = sum_c WpT[c, d] * q[c, n]  (+bp) ----
            projb = []
            for md in range(2):
                pj = ps.tile([128, N], F32, tag="ps")
                for k in range(8):
                    lh = wh_sb[k][:, 128 * md : 128 * (md + 1)]
                    ll = wl_sb[k][:, 128 * md : 128 * (md + 1)]
                    for h in range(2):
                        hs = slice(512 * h, 512 * (h + 1))
                        # wh@qh + wh@ql + wl@qh (ll term ~2^-16, dropped)
                        nc.tensor.matmul(pj[:, hs], lh, qh_sb[k][:, hs],
                                         start=(k == 0), stop=False)
                        nc.tensor.matmul(pj[:, hs], lh, ql_sb[k][:, hs],
                                         start=False, stop=False)
                        nc.tensor.matmul(pj[:, hs], ll, qh_sb[k][:, hs],
                                         start=False, stop=(k == 7))
                pb = stt.tile([128, N], F32, tag=f"pb{md}")
                if simple:
                    nc.vector.tensor_copy(pb[:], pj[:])
                else:
                    nc.vector.tensor_scalar_add(pb[:], pj[:], bp_sb[:, md : md + 1])
                projb.append(pb)

            # squares (ScalarE, keeps DVE free)
            sq = []
            for md in range(2):
                s = wrk.tile([128, N], F32, tag=f"sq{md}")
                nc.scalar.square(s[:], projb[md][:])
                sq.append(s)

            # stats via ones-matmuls: mean & E[v^2], broadcast to 128 partitions
            smq = ps.tile([128, N], F32, tag="ps")
            for md in range(2):
                for h in range(2):
                    nc.tensor.matmul(
                        smq[:, 512 * h : 512 * (h + 1)],
                        mm_dt(cq_sb[:], f32r_stats),
                        mm_dt(projb[md][:, 512 * h : 512 * (h + 1)], f32r_stats),
                        start=(md == 0),
                        stop=(md == 1),
                    )
            sqq = ps.tile([128, N], F32, tag="ps")
            for md in range(2):
                for h in range(2):
                    nc.tensor.matmul(
                        sqq[:, 512 * h : 512 * (h + 1)],
                        mm_dt(cq_sb[:], f32r_stats),
                        mm_dt(sq[md][:, 512 * h : 512 * (h + 1)], f32r_stats),
                        start=(md == 0),
                        stop=(md == 1),
                    )

            mb = stt.tile([128, N], F32, tag="mb")
            nc.vector.tensor_copy(mb[:], smq[:])
            m2 = wrk.tile([128, N], F32, tag="t")
            nc.scalar.square(m2[:], mb[:])
            var = wrk.tile([128, N], F32, tag="t2")
            nc.vector.tensor_sub(var[:], sqq[:], m2[:])
            sd = wrk.tile([128, N], F32, tag="t")
            nc.scalar.activation(sd[:], var[:], SQRT, bias=eps_t[:])
            rstd = stt.tile([128, N], F32, tag="rstd")
            rscr = wrk.tile([128, N], F32, tag="t3")
            nc.vector.reciprocal_approx_accurate(rstd[:], sd[:], rscr[:])

            # simple mode: qn holds (projb - mean); rstd is folded into XR so the
            # per-tile multiply produces (projb-m)*rstd*xn in one op.
            qn = []
            qn_lo = []
            for md in range(2):
                qq = stt.tile([128, N], F32, tag=f"qn{md}")
                nc.vector.tensor_sub(qq[:], projb[md][:], mb[:])
                if not simple:
                    nc.vector.tensor_mul(qq[:], qq[:], rstd[:])
                    nc.vector.tensor_scalar(
                        qq[:], qq[:], g1_sb[:, md : md + 1], b1_sb[:, md : md + 1],
                        op0=MULT, op1=ADD,
                    )
                if split:
                    # bf16 hi/lo decomposition: qq = hi + lo, |lo| <~ 2^-8 |qq|
                    qh = stt.tile([128, N], BF16, tag=f"qh{md}")
                    nc.vector.tensor_copy(qh[:], qq[:])
                    ql = stt.tile([128, N], BF16, tag=f"ql{md}")
                    nc.vector.tensor_sub(ql[:], qq[:], qh[:])
                    qn.append(qh)
                    qn_lo.append(ql)
                else:
                    qn.append(qq)

            # ---- x LayerNorm (over 32 channels) + partition replication ----
            xsq = wrk.tile([C2, N], F32, tag="xq")
            nc.scalar.square(xsq[:], x_sb[:])
            smx = ps.tile([128, N], F32, tag="ps")
            for h in range(2):
                nc.tensor.matmul(
                    smx[:, 512 * h : 512 * (h + 1)], mm_dt(cx_sb[:], f32r_stats),
                    mm_dt(x_sb[:, 512 * h : 512 * (h + 1)], f32r_stats),
                    start=True, stop=True,
                )
            sqx = ps.tile([128, N], F32, tag="ps")
            for h in range(2):
                nc.tensor.matmul(
                    sqx[:, 512 * h : 512 * (h + 1)], mm_dt(cx_sb[:], f32r_stats),
                    mm_dt(xsq[:, 512 * h : 512 * (h + 1)], f32r_stats),
                    start=True, stop=True,
                )
            xb = ps.tile([128, N], F32, tag="ps")
            for h in range(2):
                nc.tensor.matmul(
                    xb[:, 512 * h : 512 * (h + 1)],
                    mm_dt(sx_sb[:], f32r_sel),
                    mm_dt(x_sb[:, 512 * h : 512 * (h + 1)], f32r_sel),
                    start=True, stop=True,
                )

            mxb = wrk.tile([128, N], F32, tag="mx")
            nc.vector.tensor_copy(mxb[:], smx[:])
            mx2 = wrk.tile([128, N], F32, tag="t")
            nc.scalar.square(mx2[:], mxb[:])
            vx = wrk.tile([128, N], F32, tag="t2")
            nc.vector.tensor_sub(vx[:], sqx[:], mx2[:])
            sdx = wrk.tile([128, N], F32, tag="t")
            nc.scalar.activation(sdx[:], vx[:], SQRT, bias=eps_t[:])
            rsx = wrk.tile([128, N], F32, tag="t3")
            rscx = wrk.tile([128, N], F32, tag="t4")
            nc.vector.reciprocal_approx_accurate(rsx[:], sdx[:], rscx[:])
            xt = wrk.tile([128, N], F32, tag="t2")
            nc.vector.tensor_sub(xt[:], xb[:], mxb[:])
            xnr = stt.tile([128, N], F32, tag="xnr")
            nc.vector.tensor_mul(xnr[:], xt[:], rsx[:])
            if simple:
                # fold q-side rstd into the shared multiplier tile
                nc.vector.tensor_mul(xnr[:], xnr[:], rstd[:])
            else:
                nc.vector.tensor_scalar(
                    xnr[:], xnr[:], g2_sb[:, 0:1], b2_sb[:, 0:1], op0=MULT, op1=ADD
                )

            # ---- outer product: 64 output tiles of [128, 1024] ----
            # tile t = (md, g, r): output rows 128t..128(t+1), qn rows
            # 128md + 32g + 4r + {0..3}. lhsT and rhs share base partition 32g
            # (tile_position constraint); rep_sb holds the selection matrices
            # replicated vertically 4x so any 32-row slice works.
            out_dma_engines = [nc.sync, nc.scalar]
            ot = None
            for md in range(2):
                for g in range(2):
                    for r in range(16):
                        qb = ps.tile([128, N], F32, tag="ps")
                        lhsT = rep_sb[64 * g : 64 * (g + 1), 128 * r : 128 * (r + 1)]
                        for h in range(2):
                            if split:
                                nc.tensor.matmul(
                                    qb[:, 512 * h : 512 * (h + 1)],
                                    lhsT,
                                    qn[md][64 * g : 64 * (g + 1), 512 * h : 512 * (h + 1)],
                                    start=True,
                                    stop=False,
                                )
                                nc.tensor.matmul(
                                    qb[:, 512 * h : 512 * (h + 1)],
                                    lhsT,
                                    qn_lo[md][64 * g : 64 * (g + 1), 512 * h : 512 * (h + 1)],
                                    start=False,
                                    stop=True,
                                )
                            else:
                                nc.tensor.matmul(
                                    qb[:, 512 * h : 512 * (h + 1)],
                                    mm_dt(lhsT, f32r_sel),
                                    mm_dt(qn[md][64 * g : 64 * (g + 1), 512 * h : 512 * (h + 1)], f32r_sel),
                                    start=True,
                                    stop=True,
                                )
                        t = md * 32 + g * 16 + r
                        if t % 2 == 0:
                            ot = ost.tile([128, 2 * N], F32)
                        nc.vector.tensor_mul(
                            ot[:, (t % 2) * N : (t % 2 + 1) * N], qb[:], xnr[:]
                        )
                        if t % 2 == 1:
                            eng = out_dma_engines[(t // 2) % 2]
                            # DRAM rows 128(t-1)+p (half 0) and 128t+p (half 1)
                            # must match SBUF partition p's two 1024-col halves.
                            dst = out_d[128 * (t - 1) : 128 * (t + 1), :].rearrange(
                                "(h p) n -> p h n", h=2
                            )
                            src = ot[:].rearrange("p (h n) -> p h n", h=2)
                            eng.dma_start(dst, src)

    nc.compile()
    return nc


def _host_inputs(q, x, Wp, bp, g1, b1, g2, b2):
    """Build the 8 per-core input maps."""
    import os

    import ml_dtypes
    qf = np.ascontiguousarray(np.asarray(q, dtype=np.float32).reshape(B, C1, N))
    qfh = qf.astype(ml_dtypes.bfloat16)
    qfl = (qf - qfh.astype(np.float32)).astype(ml_dtypes.bfloat16)
    xf = np.ascontiguousarray(np.asarray(x, dtype=np.float32).reshape(B, C2, N))
    wpt = np.ascontiguousarray(np.asarray(Wp, dtype=np.float32).T)
    wh = wpt.astype(ml_dtypes.bfloat16)
    wl = (wpt - wh.astype(np.float32)).astype(ml_dtypes.bfloat16)
    bpc = np.ascontiguousarray(np.asarray(bp, dtype=np.float32).reshape(2, 128).T)
    g1c = np.ascontiguousarray(np.asarray(g1, dtype=np.float32).reshape(2, 128).T)
    b1c = np.ascontiguousarray(np.asarray(b1, dtype=np.float32).reshape(2, 128).T)
    g2r = np.ascontiguousarray(np.tile(np.asarray(g2, dtype=np.float32), 4)[:, None])
    b2r = np.ascontiguousarray(np.tile(np.asarray(b2, dtype=np.float32), 4)[:, None])
    # rep[:, r*128+p]: vertical 2x stack of S64_r, S64_r[k,p] = d(k, 4r + p//32)
    rep = np.zeros((128, 16 * 128), dtype=np.float32)
    for r in range(16):
        for p in range(128):
            k = 4 * r + p // 32
            for v in range(2):
                rep[64 * v + k, 128 * r + p] = 1.0
    if os.environ.get("HM_SPLIT", "1") == "1":
        rep = rep.astype(ml_dtypes.bfloat16)
    sx = np.zeros((C2, 128), dtype=np.float32)
    for p in range(128):
        sx[p % 32, p] = 1.0
    in_maps = []
    for b in range(B):
        in_maps.append(
            {
                "qh": np.ascontiguousarray(qfh[b]),
                "ql": np.ascontiguousarray(qfl[b]),
                "x": xf[b],
                "wh": wh,
                "wl": wl,
                "bpc": bpc,
                "g1c": g1c,
                "b1c": b1c,
                "g2r": g2r,
                "b2r": b2r,
                "rep": rep,
                "sx": sx,
            }
        )
    return in_maps


def _run(in_maps, trace=False):
    import os

    from concourse.bass_utils import run_bass_kernel_spmd

    key = "nc" + os.environ.get("HM_SIMPLE", "0")
    if key not in _CACHE:
        _CACHE[key] = _build_nc()
    nc = _CACHE[key]
    res = run_bass_kernel_spmd(
        nc, in_maps, core_ids=list(range(B)), trace=trace
    )
    return res


def kernel(q, x, Wp, bp, g1, b1, g2, b2):
    import os

    simple = (
        np.allclose(np.asarray(bp), 0)
        and np.allclose(np.asarray(g1), 1)
        and np.allclose(np.asarray(b1), 0)
        and np.allclose(np.asarray(g2), 1)
        and np.allclose(np.asarray(b2), 0)
    )
    os.environ["HM_SIMPLE"] = "1" if simple else "0"
    in_maps = _host_inputs(q, x, Wp, bp, g1, b1, g2, b2)
    res = _run(in_maps, trace=False)
    out = np.stack(
        [res.results[b]["out"].reshape(CD, H, W) for b in range(B)]
    ).astype(np.float32)
    _CACHE["last_res"] = res
    return out



# revision 23
# speedup vs baseline: 2.1088x; 2.1088x over previous
"""Trainium2 Bass kernel for nn_HadaMard: fused proj + 2xLayerNorm + outer product.

Reference computation (per batch b):
  qf = q[b].reshape(C1, N)           # [1024, 1024]  (C1 rows, N=H*W cols)
  proj = Wp @ qf + bp                # [256, 1024]
  qn = LN_over_d(proj) * g1 + b1     # LN over the 256-channel dim
  xn = LN_over_e(x[b]) * g2 + b2     # LN over the 32-channel dim
  out[d*32+e, n] = qn[d, n] * xn[e, n]   # [8192, 1024]

Sharding: data-parallel over B=8, one batch per NeuronCore.

Layout ("flipped tiling"): output tiles keep qn's channel dim d on the
partitions (dblock in {0,1} x 128 partitions) and iterate e in the free dim.
  - proj: PE matmuls (bf16), accumulated in f32 PSUM, k-loop ordered by
    DMA arrival; the q-stats/LN chain is pipelined by 512-column halves
    so qn's first half is ready early.
  - LN stats via bf16 ones-matmuls; 1/sd via reciprocal_approx_fast.
  - xn (32 rows, bf16) replicated to 128 partitions via DRAM-roundtrip
    DMAs with stride-0 source (partition_broadcast); one scratch copy per
    issuing engine keeps the read ordered behind the write in-queue.
  - product: all-bf16 tensor_tensor multiplies (DVE 2x mode) with the qn
    operand repeated along the free dim via a stride-0 AP; ~1/3 of the
    chunks run on the Pool engine.
  - output: bf16 DRAM tensor (host converts to f32), 4-e-wide tiles,
    DMAs spread across SP / Act / Pool.

Axon-backend constraints honored: no float32r matmuls, no AluOp.divide,
at most one PSUM operand per DVE op, no PSUM operands on Pool, DMA only
on SP / Act / Pool.
"""

import numpy as np

_CACHE = {}

B, C1, H, W = 8, 1024, 32, 32
C2 = 32
Cp = 256
N = 1024
CD = Cp * C2  # 8192
MD = Cp // 128  # 2 row-blocks of proj/qn
EPS = 1e-5

# mul chunks (e0, e1) per dblock for DVE ('v') and Pool ('g').
# DVE chunks are emitted per column-half; Pool chunks are full-width.
_MUL_V = {
    0: [(0, 2), (2, 4), (4, 8), (8, 12), (12, 16), (16, 20), (20, 24),
        (24, 28), (28, 32)],
    1: [(12, 16), (16, 20), (20, 24)],
}
_MUL_G = {
    0: [],
    1: [(0, 2), (2, 4), (4, 6), (6, 8), (8, 10), (10, 12), (24, 26), (26, 28),
        (28, 30), (30, 32)],
}
# xn broadcast chunks: (e0, e1, engine): 's' SP, 'a' Act, 'g' Pool
_BCAST = [(0, 2, "s"), (2, 4, "s"), (4, 8, "s"), (8, 12, "s"), (12, 16, "s"),
          (16, 20, "a"), (20, 24, "a"), (24, 32, "g")]
# output tiles per dblock: 8 x 4-e tiles, (j -> dma engine)
_OUT_ENG = {
    0: ["s", "a", "s", "a", "s", "a", "s", "a"],
    1: ["a", "g", "s", "a", "g", "a", "g", "s"],
}
# O-tile allocation order (rough completion order; pool bufs=6)
_ALLOC_ORDER = [(0, 0), (0, 1), (1, 0), (0, 2), (1, 1), (0, 3), (1, 2),
                (1, 3), (0, 4), (1, 4), (0, 5), (1, 5), (0, 6), (1, 6),
                (0, 7), (1, 7)]
# q-tile k -> load engine; proj accumulation follows arrival order
_Q_ENG = {0: "s", 3: "s", 1: "a", 4: "a", 2: "g", 5: "g", 6: "g", 7: "g"}
_K_ORDER = [2, 0, 1, 5, 3, 4, 6, 7]


def _build_nc(simple):
    import concourse.bacc as bacc
    import concourse.bass as bass
    import concourse.mybir as mybir
    import concourse.tile as tile

    F32 = mybir.dt.float32
    BF16 = mybir.dt.bfloat16
    MULT = mybir.AluOpType.mult
    SUB = mybir.AluOpType.subtract
    ADD = mybir.AluOpType.add
    SQRT = mybir.ActivationFunctionType.Sqrt

    nc = bacc.Bacc(None, target_bir_lowering=False)

    q_d = nc.dram_tensor("q", [C1, N], BF16, kind="ExternalInput")
    w_d = nc.dram_tensor("w", [C1, Cp], BF16, kind="ExternalInput")
    x_d = nc.dram_tensor("x", [C2, N], BF16, kind="ExternalInput")
    if not simple:
        bp_d = nc.dram_tensor("bpc", [128, MD], F32, kind="ExternalInput")
        g1_d = nc.dram_tensor("g1c", [128, MD], F32, kind="ExternalInput")
        b1_d = nc.dram_tensor("b1c", [128, MD], F32, kind="ExternalInput")
        g2_d = nc.dram_tensor("g2r", [C2, 1], F32, kind="ExternalInput")
        b2_d = nc.dram_tensor("b2r", [C2, 1], F32, kind="ExternalInput")
    xs_d = {
        "s": nc.dram_tensor("xs0", [C2, N], BF16, kind="ExternalOutput"),
        "a": nc.dram_tensor("xs1", [C2, N], BF16, kind="ExternalOutput"),
        "g": nc.dram_tensor("xs2", [C2, N], BF16, kind="ExternalOutput"),
    }
    out_d = nc.dram_tensor("out", [CD, N], BF16, kind="ExternalOutput")

    def rep_ap(t, r, h=None):
        """qn tile AP repeated r times along a stride-0 free dim.

        h=None: full rows; h=0/1: 512-column half (offset 512h)."""
        a = t[:].copy()
        while len(a.ap) > 0:
            a.ap.pop()
        a.ap.append([N, 128])
        a.ap.append([0, r])
        if h is None:
            a.ap.append([1, N])
        else:
            a.ap.append([1, 512])
            a.offset = a.offset + 512 * h
        return a

    def sub_ap(t, e0, e1, h=None):
        """e-major tile viewed as (p, e, n): slice e and optionally a
        512-col half of n.  Partition stride taken from the tile itself."""
        a = t[:].copy()
        base = a.offset
        pstride = a.ap[0][0]
        while len(a.ap) > 0:
            a.ap.pop()
        a.ap.append([pstride, 128])
        a.ap.append([N, e1 - e0])
        if h is None:
            a.ap.append([1, N])
            a.offset = base + e0 * N
        else:
            a.ap.append([1, 512])
            a.offset = base + e0 * N + 512 * h
        return a

    with tile.TileContext(nc) as tc:
        with (
            tc.tile_pool(name="cst", bufs=1) as cst,
            tc.tile_pool(name="qp", bufs=1) as qp,
            tc.tile_pool(name="wrk", bufs=1) as wrk,
            tc.tile_pool(name="bp16", bufs=1) as bp16,
            tc.tile_pool(name="keep", bufs=1) as keep,
            tc.tile_pool(name="xbe", bufs=1) as xbep,
            tc.tile_pool(name="op", bufs=8) as op,
            tc.tile_pool(name="ps", bufs=4, space=bass.MemorySpace.PSUM) as ps,
        ):
            eng = {"s": nc.sync, "a": nc.scalar, "g": nc.gpsimd}

            # ---------- constants / memsets (DVE) ----------
            onesx = cst.tile([C2, C2], BF16, tag="onesx")
            nc.vector.memset(onesx[:], 1.0 / C2)
            onesq = cst.tile([128, 128], BF16, tag="onesq")
            nc.vector.memset(onesq[:], 1.0 / Cp)
            eps_t = cst.tile([128, 1], F32, tag="eps")
            nc.vector.memset(eps_t[:], EPS)

            _wn = [0]

            def wtile():
                t = wrk.tile([128, N], F32, tag=f"t{_wn[0] % 6}")
                _wn[0] += 1
                return t

            halves = [slice(0, 512), slice(512, 1024)]

            # ---------- input loads ----------
            # SP: w0, q0, x, q3; Act: w1, q1, q4; Pool: q2, q5, q6, q7
            wg = []
            for g in range(2):
                t = cst.tile([128, 4 * Cp], BF16, tag=f"w{g}")
                dst = t[:].rearrange("p (k d) -> p k d", k=4)
                src = w_d[512 * g : 512 * (g + 1), :].rearrange(
                    "(k p) d -> p k d", k=4
                )
                [nc.sync, nc.scalar][g].dma_start(dst, src)
                wg.append(t)
            xsb = cst.tile([C2, N], BF16, tag="x")
            nc.sync.dma_start(xsb[:], x_d[:])
            q_sb = {}
            for k in [2, 0, 1, 5, 3, 4, 6, 7]:
                t = qp.tile([128, N], BF16, tag=f"q{k}")
                eng[_Q_ENG[k]].dma_start(t[:], q_d[128 * k : 128 * (k + 1), :])
                q_sb[k] = t
            if not simple:
                bp_sb = cst.tile([128, MD], F32, tag="bp")
                nc.sync.dma_start(bp_sb[:], bp_d[:])
                g1_sb = cst.tile([128, MD], F32, tag="g1")
                nc.sync.dma_start(g1_sb[:], g1_d[:])
                b1_sb = cst.tile([128, MD], F32, tag="b1")
                nc.scalar.dma_start(b1_sb[:], b1_d[:])
                g2_sb = cst.tile([C2, 1], F32, tag="g2")
                nc.scalar.dma_start(g2_sb[:], g2_d[:])
                b2_sb = cst.tile([C2, 1], F32, tag="b2")
                nc.scalar.dma_start(b2_sb[:], b2_d[:])

            # ---------- x stats + first part of proj (PE) ----------
            xsq = bp16.tile([C2, N], BF16, tag="xsq")
            nc.vector.tensor_tensor(xsq[:], xsb[:], xsb[:], op=MULT)
            mx_ps = ps.tile([128, N], F32, tag="ps")
            mxq_ps = ps.tile([128, N], F32, tag="ps")

            proj = []
            for md in range(MD):
                pj = ps.tile([128, N], F32, tag="ps")
                proj.append(pj)

            def proj_mms(i, k):
                for hs in halves:
                    for md in range(MD):
                        lh = wg[k // 4][:, (k % 4) * Cp + 128 * md :
                                        (k % 4) * Cp + 128 * (md + 1)]
                        nc.tensor.matmul(proj[md][:, hs], lh, q_sb[k][:, hs],
                                         start=(i == 0), stop=(i == 7))

            # PE queue: proj[k2, k0], x-stat mms, proj[rest]
            proj_mms(0, _K_ORDER[0])
            proj_mms(1, _K_ORDER[1])
            for hs in halves:
                nc.tensor.matmul(mx_ps[:C2, hs], onesx[:], xsb[:, hs],
                                 start=True, stop=True)
            for hs in halves:
                nc.tensor.matmul(mxq_ps[:C2, hs], onesx[:], xsq[:, hs],
                                 start=True, stop=True)
            for i in range(2, 8):
                proj_mms(i, _K_ORDER[i])

            # ---------- x LN (half-pipelined chain) ----------
            mx2 = wtile()
            xd = wtile()
            varx = wtile()
            sdx = wtile()
            rsdx = keep.tile([C2, N], F32, tag="rsdx")
            xn = keep.tile([C2, N], BF16, tag="xn")
            if not simple:
                xtmp = wtile()
            for hs in halves:
                nc.scalar.square(mx2[:C2, hs], mx_ps[:C2, hs])
                nc.vector.tensor_tensor(xd[:C2, hs], xsb[:, hs],
                                        mx_ps[:C2, hs], op=SUB)
                nc.vector.tensor_tensor(varx[:C2, hs], mxq_ps[:C2, hs],
                                        mx2[:C2, hs], op=SUB)
                nc.scalar.activation(sdx[:C2, hs], varx[:C2, hs], SQRT,
                                     bias=eps_t[:C2, :])
                nc.vector.reciprocal_approx_fast(rsdx[:, hs], sdx[:C2, hs])
                if simple:
                    nc.vector.tensor_tensor(xn[:, hs], xd[:C2, hs],
                                            rsdx[:, hs], op=MULT)
                else:
                    nc.vector.tensor_tensor(xtmp[:C2, hs], xd[:C2, hs],
                                            rsdx[:, hs], op=MULT)
                    nc.vector.tensor_scalar(xn[:, hs], xtmp[:C2, hs],
                                            g2_sb[:], b2_sb[:],
                                            op0=MULT, op1=ADD)

            # ---------- xn scratch writes + stride-0 broadcasts ----------
            xbe = {}
            _bc_written = set()

            def emit_bcast(which):
                for e0, e1, en in _BCAST:
                    if en != which:
                        continue
                    if which not in _bc_written:
                        eng[which].dma_start(xs_d[which][:], xn[:])
                        _bc_written.add(which)
                    t = xbep.tile([128, (e1 - e0) * N], BF16,
                                  tag=f"xbe{e0}")
                    eng[which].dma_start(
                        t[:], xs_d[which][e0:e1, :].partition_broadcast(128))
                    xbe[(e0, e1)] = t

            emit_bcast("s")
            emit_bcast("g")

            # ---------- q stats, pipelined by 512-column halves ----------
            pb, sq, diff, qnb, dvk = [], [], [], [], []
            for md in range(MD):
                pbt = bp16.tile([128, N], BF16, tag=f"pb{md}")
                pb.append(pbt)
                sqt = bp16.tile([128, N], BF16, tag=f"sq{md}")
                sq.append(sqt)
            mean_ps = ps.tile([128, N], F32, tag="ps")
            msq_ps = ps.tile([128, N], F32, tag="ps")
            mb2 = wtile()
            var = wtile()
            sd = wtile()
            for md in range(MD):
                dft = wtile()
                diff.append(dft)
            rsd = keep.tile([128, N], F32, tag="rsd")
            for md in range(MD):
                qnt = keep.tile([128, N], BF16, tag=f"qn{md}")
                qnb.append(qnt)
            qnb_pool = keep.tile([128, N], BF16, tag="qnp")
            if not simple:
                for md in range(MD):
                    dvt = keep.tile([128, N], F32, tag=f"dv{md}")
                    dvk.append(dvt)

            # ---------- product helpers ----------
            def xbe_of(e0, e1):
                for (b0, b1), t in xbe.items():
                    if b0 <= e0 and e1 <= b1:
                        return t, b0
                raise AssertionError((e0, e1))

            out_view = []
            for md in range(MD):
                ov = out_d[4096 * md : 4096 * (md + 1), :].rearrange(
                    "(p e) n -> p e n", e=32
                )
                out_view.append(ov)

            otile = {}
            for md, j in _ALLOC_ORDER:
                ot = op.tile([128, 4 * N], BF16, tag="ot")
                otile[(md, j)] = ot

            def emit_mul(e_, qsrc, md, e0, e1, h=None):
                j = e0 // 4
                assert e1 <= 4 * (j + 1)
                o = otile[(md, j)]
                xt, b0 = xbe_of(e0, e1)
                e_.tensor_tensor(
                    sub_ap(o, e0 - 4 * j, e1 - 4 * j, h)
                    if h is not None
                    else o[:, (e0 - 4 * j) * N : (e1 - 4 * j) * N],
                    rep_ap(qsrc, e1 - e0, h),
                    sub_ap(xt, e0 - b0, e1 - b0, h),
                    op=MULT)

            def emit_out(md, j):
                o = otile[(md, j)]
                eng[_OUT_ENG[md][j]].dma_start(
                    out_view[md][:, 4 * j : 4 * (j + 1), :], o[:])

            # DVE mul order within a half: by e (broadcast arrival order)
            vseq = sorted(
                [(0, c) for c in _MUL_V[0]] + [(1, c) for c in _MUL_V[1]],
                key=lambda mc: (mc[1][0], mc[0]))

            # ---------- stats chain + muls, interleaved per half ----------
            for hi, hs in enumerate(halves):
                for md in range(MD):
                    if simple:
                        nc.scalar.copy(pb[md][:, hs], proj[md][:, hs])
                    else:
                        nc.vector.tensor_scalar(pb[md][:, hs],
                                                proj[md][:, hs],
                                                bp_sb[:, md : md + 1], None,
                                                op0=ADD)
                    # squares from the bf16 copies: md0 DVE, md1 Pool
                    e_ = nc.vector if md == 0 else nc.gpsimd
                    e_.tensor_tensor(sq[md][:, hs], pb[md][:, hs],
                                     pb[md][:, hs], op=MULT)
                for md in range(MD):
                    nc.tensor.matmul(mean_ps[:, hs], onesq[:], pb[md][:, hs],
                                     start=(md == 0), stop=(md == MD - 1))
                for md in range(MD):
                    nc.tensor.matmul(msq_ps[:, hs], onesq[:], sq[md][:, hs],
                                     start=(md == 0), stop=(md == MD - 1))
                nc.scalar.square(mb2[:, hs], mean_ps[:, hs])
                nc.vector.tensor_tensor(diff[0][:, hs], pb[0][:, hs],
                                        mean_ps[:, hs], op=SUB)
                nc.vector.tensor_tensor(var[:, hs], msq_ps[:, hs],
                                        mb2[:, hs], op=SUB)
                nc.scalar.activation(sd[:, hs], var[:, hs], SQRT,
                                     bias=eps_t[:])
                nc.vector.tensor_tensor(diff[1][:, hs], pb[1][:, hs],
                                        mean_ps[:, hs], op=SUB)
                nc.vector.reciprocal_approx_fast(rsd[:, hs], sd[:, hs])
                if simple:
                    nc.vector.tensor_tensor(qnb[0][:, hs], diff[0][:, hs],
                                            rsd[:, hs], op=MULT)
                    nc.vector.tensor_tensor(qnb[1][:, hs], diff[1][:, hs],
                                            rsd[:, hs], op=MULT)
                    nc.gpsimd.tensor_tensor(qnb_pool[:, hs], diff[1][:, hs],
                                            rsd[:, hs], op=MULT)
                else:
                    for md in range(MD):
                        nc.vector.tensor_tensor(dvk[md][:, hs],
                                                diff[md][:, hs],
                                                rsd[:, hs], op=MULT)
                        nc.vector.tensor_scalar(qnb[md][:, hs],
                                                dvk[md][:, hs],
                                                g1_sb[:, md : md + 1],
                                                b1_sb[:, md : md + 1],
                                                op0=MULT, op1=ADD)
                    nc.gpsimd.tensor_copy(qnb_pool[:, hs], qnb[1][:, hs])
                # Act-side broadcasts between the halves: the h1-stats
                # delay they cause is hidden behind DVE's h0 mul backlog
                if hi == 0:
                    emit_bcast("a")
                # this half's muls (high priority: the scheduler should
                # prefer them over later chain ops whenever they are ready)
                with tc.high_priority():
                    for e0, e1 in _MUL_G[1]:
                        emit_mul(nc.gpsimd, qnb_pool, 1, e0, e1, hi)
                    for md, (e0, e1) in vseq:
                        emit_mul(nc.vector, qnb[md], md, e0, e1, hi)

            # output DMAs
            for md, j in _ALLOC_ORDER:
                emit_out(md, j)

    nc.compile()
    return nc


def _host_inputs(q, x, Wp, bp, g1, b1, g2, b2):
    """Build the 8 per-core input maps."""
    import os

    import ml_dtypes

    simple = os.environ.get("HM_SIMPLE", "0") == "1"
    qf = np.asarray(q, dtype=np.float32).reshape(B, C1, N)
    qb = np.ascontiguousarray(qf).astype(ml_dtypes.bfloat16)
    xf = np.ascontiguousarray(
        np.asarray(x, dtype=np.float32).reshape(B, C2, N)
    ).astype(ml_dtypes.bfloat16)
    wpt = np.ascontiguousarray(np.asarray(Wp, dtype=np.float32).T).astype(
        ml_dtypes.bfloat16
    )
    in_maps = []
    for b in range(B):
        m = {
            "q": np.ascontiguousarray(qb[b]),
            "w": wpt,
            "x": np.ascontiguousarray(xf[b]),
        }
        if not simple:
            m["bpc"] = np.ascontiguousarray(
                np.asarray(bp, dtype=np.float32).reshape(MD, 128).T)
            m["g1c"] = np.ascontiguousarray(
                np.asarray(g1, dtype=np.float32).reshape(MD, 128).T)
            m["b1c"] = np.ascontiguousarray(
                np.asarray(b1, dtype=np.float32).reshape(MD, 128).T)
            m["g2r"] = np.ascontiguousarray(
                np.asarray(g2, dtype=np.float32)[:, None])
            m["b2r"] = np.ascontiguousarray(
                np.asarray(b2, dtype=np.float32)[:, None])
        in_maps.append(m)
    return in_maps


def _run(in_maps, trace=False):
    import os

    from concourse.bass_utils import run_bass_kernel_spmd

    key = "nc" + os.environ.get("HM_SIMPLE", "0")
    if key not in _CACHE:
        _CACHE[key] = _build_nc(os.environ.get("HM_SIMPLE", "0") == "1")
    nc = _CACHE[key]
    res = run_bass_kernel_spmd(nc, in_maps, core_ids=list(range(B)), trace=trace)
    return res


def kernel(q, x, Wp, bp, g1, b1, g2, b2):
    import os

    simple = (
        np.allclose(np.asarray(bp), 0)
        and np.allclose(np.asarray(g1), 1)
        and np.allclose(np.asarray(b1), 0)
        and np.allclose(np.asarray(g2), 1)
        and np.allclose(np.asarray(b2), 0)
    )
    os.environ["HM_SIMPLE"] = "1" if simple else "0"
    in_maps = _host_inputs(q, x, Wp, bp, g1, b1, g2, b2)
    res = _run(in_maps, trace=False)
    out = np.stack(
        [
            np.asarray(res.results[b]["out"]).astype(np.float32).reshape(CD, H, W)
            for b in range(B)
        ]
    )
    _CACHE["last_res"] = res
    return out


# revision 26
# speedup vs baseline: 2.1365x; 1.0131x over previous
"""Trainium2 Bass kernel for nn_HadaMard: fused proj + 2xLayerNorm + outer product.

Reference computation (per batch b):
  qf = q[b].reshape(C1, N)           # [1024, 1024]  (C1 rows, N=H*W cols)
  proj = Wp @ qf + bp                # [256, 1024]
  qn = LN_over_d(proj) * g1 + b1     # LN over the 256-channel dim
  xn = LN_over_e(x[b]) * g2 + b2     # LN over the 32-channel dim
  out[d*32+e, n] = qn[d, n] * xn[e, n]   # [8192, 1024]

Sharding: data-parallel over B=8, one batch per NeuronCore.

Layout ("flipped tiling"): output tiles keep qn's channel dim d on the
partitions (dblock in {0,1} x 128 partitions) and iterate e in the free dim.
  - proj: PE matmuls (bf16), accumulated in f32 PSUM, k-loop ordered by
    DMA arrival; the q-stats/LN chain is pipelined by 512-column halves
    so qn's first half is ready early.
  - LN stats via bf16 ones-matmuls; 1/sd via reciprocal_approx_fast.
  - xn (32 rows, bf16) replicated to 128 partitions via DRAM-roundtrip
    DMAs with stride-0 source (partition_broadcast); one scratch copy per
    issuing engine keeps the read ordered behind the write in-queue.
  - product: all-bf16 tensor_tensor multiplies (DVE 2x mode) with the qn
    operand repeated along the free dim via a stride-0 AP; ~1/3 of the
    chunks run on the Pool engine.
  - output: bf16 DRAM tensor (host converts to f32), 4-e-wide tiles,
    DMAs spread across SP / Act / Pool.

Axon-backend constraints honored: no float32r matmuls, no AluOp.divide,
at most one PSUM operand per DVE op, no PSUM operands on Pool, DMA only
on SP / Act / Pool.
"""

import numpy as np

_CACHE = {}

B, C1, H, W = 8, 1024, 32, 32
C2 = 32
Cp = 256
N = 1024
CD = Cp * C2  # 8192
MD = Cp // 128  # 2 row-blocks of proj/qn
EPS = 1e-5

# mul chunks (e0, e1) per dblock for DVE ('v') and Pool ('g').
# DVE chunks are emitted per column-half; Pool chunks are full-width.
_MUL_V = {
    0: [(0, 2), (2, 4), (4, 8), (8, 12), (12, 16), (16, 20), (20, 24),
        (24, 28), (28, 32)],
    1: [(12, 16), (16, 20), (20, 24)],
}
_MUL_G = {
    0: [],
    1: [(0, 2), (2, 4), (4, 6), (6, 8), (8, 10), (10, 12), (24, 26), (26, 28),
        (28, 30), (30, 32)],
}
# xn broadcast chunks: (e0, e1, engine): 's' SP, 'a' Act, 'g' Pool
_BCAST = [(0, 2, "s"), (2, 4, "s"), (4, 8, "s"), (8, 12, "s"), (12, 16, "s"),
          (16, 20, "a"), (20, 24, "a"), (24, 32, "g")]
# output tiles per dblock: 8 x 4-e tiles, (j -> dma engine)
_OUT_ENG = {
    0: ["s", "a", "s", "a", "s", "a", "s", "a"],
    1: ["a", "g", "s", "a", "g", "a", "g", "s"],
}
# O-tile allocation order (rough completion order; pool bufs=6)
_ALLOC_ORDER = [(0, 0), (0, 1), (1, 0), (0, 2), (1, 1), (0, 3), (1, 2),
                (1, 3), (0, 4), (1, 4), (0, 5), (1, 5), (0, 6), (1, 6),
                (0, 7), (1, 7)]
# q-tile k -> load engine; proj accumulation follows arrival order
_Q_ENG = {0: "s", 3: "s", 1: "a", 4: "a", 2: "g", 5: "g", 6: "g", 7: "g"}
_K_ORDER = [2, 0, 1, 5, 3, 4, 6, 7]


def _build_nc(simple):
    import concourse.bacc as bacc
    import concourse.bass as bass
    import concourse.mybir as mybir
    import concourse.tile as tile

    F32 = mybir.dt.float32
    BF16 = mybir.dt.bfloat16
    MULT = mybir.AluOpType.mult
    SUB = mybir.AluOpType.subtract
    ADD = mybir.AluOpType.add
    SQRT = mybir.ActivationFunctionType.Sqrt

    nc = bacc.Bacc(None, target_bir_lowering=False)

    q_d = nc.dram_tensor("q", [C1, N], BF16, kind="ExternalInput")
    w_d = nc.dram_tensor("w", [C1, Cp], BF16, kind="ExternalInput")
    x_d = nc.dram_tensor("x", [C2, N], BF16, kind="ExternalInput")
    if not simple:
        bp_d = nc.dram_tensor("bpc", [128, MD], F32, kind="ExternalInput")
        g1_d = nc.dram_tensor("g1c", [128, MD], F32, kind="ExternalInput")
        b1_d = nc.dram_tensor("b1c", [128, MD], F32, kind="ExternalInput")
        g2_d = nc.dram_tensor("g2r", [C2, 1], F32, kind="ExternalInput")
        b2_d = nc.dram_tensor("b2r", [C2, 1], F32, kind="ExternalInput")
    xs_d = {
        "s": nc.dram_tensor("xs0", [C2, N], BF16, kind="ExternalOutput"),
        "a": nc.dram_tensor("xs1", [C2, N], BF16, kind="ExternalOutput"),
        "g": nc.dram_tensor("xs2", [C2, N], BF16, kind="ExternalOutput"),
    }
    out_d = nc.dram_tensor("out", [CD, N], BF16, kind="ExternalOutput")

    def rep_ap(t, r, h=None):
        """qn tile AP repeated r times along a stride-0 free dim.

        h=None: full rows; h=0/1: 512-column half (offset 512h)."""
        a = t[:].copy()
        while len(a.ap) > 0:
            a.ap.pop()
        a.ap.append([N, 128])
        a.ap.append([0, r])
        if h is None:
            a.ap.append([1, N])
        else:
            a.ap.append([1, 512])
            a.offset = a.offset + 512 * h
        return a

    def sub_ap(t, e0, e1, h=None):
        """e-major tile viewed as (p, e, n): slice e and optionally a
        512-col half of n.  Partition stride taken from the tile itself."""
        a = t[:].copy()
        base = a.offset
        pstride = a.ap[0][0]
        while len(a.ap) > 0:
            a.ap.pop()
        a.ap.append([pstride, 128])
        a.ap.append([N, e1 - e0])
        if h is None:
            a.ap.append([1, N])
            a.offset = base + e0 * N
        else:
            a.ap.append([1, 512])
            a.offset = base + e0 * N + 512 * h
        return a

    with tile.TileContext(nc) as tc:
        with (
            tc.tile_pool(name="cst", bufs=1) as cst,
            tc.tile_pool(name="qp", bufs=1) as qp,
            tc.tile_pool(name="wrk", bufs=1) as wrk,
            tc.tile_pool(name="bp16", bufs=1) as bp16,
            tc.tile_pool(name="keep", bufs=1) as keep,
            tc.tile_pool(name="xbe", bufs=1) as xbep,
            tc.tile_pool(name="op", bufs=8) as op,
            tc.tile_pool(name="ps", bufs=4, space=bass.MemorySpace.PSUM) as ps,
        ):
            eng = {"s": nc.sync, "a": nc.scalar, "g": nc.gpsimd}

            # ---------- constants / memsets (DVE) ----------
            onesx = cst.tile([C2, C2], BF16, tag="onesx")
            nc.vector.memset(onesx[:], 1.0 / C2)
            onesq = cst.tile([128, 128], BF16, tag="onesq")
            nc.vector.memset(onesq[:], 1.0 / Cp)
            eps_t = cst.tile([128, 1], F32, tag="eps")
            nc.vector.memset(eps_t[:], EPS)

            _wn = [0]

            def wtile():
                t = wrk.tile([128, N], F32, tag=f"t{_wn[0] % 6}")
                _wn[0] += 1
                return t

            halves = [slice(0, 512), slice(512, 1024)]

            # ---------- input loads ----------
            # SP: w0, q0, x, q3; Act: w1, q1, q4; Pool: q2, q5, q6, q7
            wg = []
            for g in range(2):
                t = cst.tile([128, 4 * Cp], BF16, tag=f"w{g}")
                dst = t[:].rearrange("p (k d) -> p k d", k=4)
                src = w_d[512 * g : 512 * (g + 1), :].rearrange(
                    "(k p) d -> p k d", k=4
                )
                [nc.sync, nc.scalar][g].dma_start(dst, src)
                wg.append(t)
            xsb = cst.tile([C2, N], BF16, tag="x")
            nc.sync.dma_start(xsb[:], x_d[:])
            q_sb = {}
            for k in [2, 0, 1, 5, 3, 4, 6, 7]:
                t = qp.tile([128, N], BF16, tag=f"q{k}")
                eng[_Q_ENG[k]].dma_start(t[:], q_d[128 * k : 128 * (k + 1), :])
                q_sb[k] = t
            if not simple:
                bp_sb = cst.tile([128, MD], F32, tag="bp")
                nc.sync.dma_start(bp_sb[:], bp_d[:])
                g1_sb = cst.tile([128, MD], F32, tag="g1")
                nc.sync.dma_start(g1_sb[:], g1_d[:])
                b1_sb = cst.tile([128, MD], F32, tag="b1")
                nc.scalar.dma_start(b1_sb[:], b1_d[:])
                g2_sb = cst.tile([C2, 1], F32, tag="g2")
                nc.scalar.dma_start(g2_sb[:], g2_d[:])
                b2_sb = cst.tile([C2, 1], F32, tag="b2")
                nc.scalar.dma_start(b2_sb[:], b2_d[:])

            # ---------- x stats + first part of proj (PE) ----------
            xsq = bp16.tile([C2, N], BF16, tag="xsq")
            nc.vector.tensor_tensor(xsq[:], xsb[:], xsb[:], op=MULT)
            mx_ps = ps.tile([128, N], F32, tag="ps")
            mxq_ps = ps.tile([128, N], F32, tag="ps")

            proj = []
            for md in range(MD):
                pj = ps.tile([128, N], F32, tag="ps")
                proj.append(pj)

            def proj_mms(i, k):
                for hs in halves:
                    for md in range(MD):
                        lh = wg[k // 4][:, (k % 4) * Cp + 128 * md :
                                        (k % 4) * Cp + 128 * (md + 1)]
                        nc.tensor.matmul(proj[md][:, hs], lh, q_sb[k][:, hs],
                                         start=(i == 0), stop=(i == 7))

            # PE queue: proj[k2, k0], x-stat mms, proj[rest]
            proj_mms(0, _K_ORDER[0])
            proj_mms(1, _K_ORDER[1])
            for hs in halves:
                nc.tensor.matmul(mx_ps[:C2, hs], onesx[:], xsb[:, hs],
                                 start=True, stop=True)
            for hs in halves:
                nc.tensor.matmul(mxq_ps[:C2, hs], onesx[:], xsq[:, hs],
                                 start=True, stop=True)
            for i in range(2, 8):
                proj_mms(i, _K_ORDER[i])

            # ---------- x LN (half-pipelined chain) ----------
            mx2 = wtile()
            xd = wtile()
            varx = wtile()
            sdx = wtile()
            rsdx = keep.tile([C2, N], F32, tag="rsdx")
            xn = keep.tile([C2, N], BF16, tag="xn")
            if not simple:
                xtmp = wtile()
            for hs in halves:
                nc.scalar.square(mx2[:C2, hs], mx_ps[:C2, hs])
                nc.vector.tensor_tensor(xd[:C2, hs], xsb[:, hs],
                                        mx_ps[:C2, hs], op=SUB)
                nc.vector.tensor_tensor(varx[:C2, hs], mxq_ps[:C2, hs],
                                        mx2[:C2, hs], op=SUB)
                nc.scalar.activation(sdx[:C2, hs], varx[:C2, hs], SQRT,
                                     bias=eps_t[:C2, :])
                nc.vector.reciprocal_approx_fast(rsdx[:, hs], sdx[:C2, hs])
                if simple:
                    nc.vector.tensor_tensor(xn[:, hs], xd[:C2, hs],
                                            rsdx[:, hs], op=MULT)
                else:
                    nc.vector.tensor_tensor(xtmp[:C2, hs], xd[:C2, hs],
                                            rsdx[:, hs], op=MULT)
                    nc.vector.tensor_scalar(xn[:, hs], xtmp[:C2, hs],
                                            g2_sb[:], b2_sb[:],
                                            op0=MULT, op1=ADD)

            # ---------- xn scratch writes + stride-0 broadcasts ----------
            xbe = {}
            _bc_written = set()

            def emit_bcast(which):
                for e0, e1, en in _BCAST:
                    if en != which:
                        continue
                    if which not in _bc_written:
                        eng[which].dma_start(xs_d[which][:], xn[:])
                        _bc_written.add(which)
                    t = xbep.tile([128, (e1 - e0) * N], BF16,
                                  tag=f"xbe{e0}")
                    eng[which].dma_start(
                        t[:], xs_d[which][e0:e1, :].partition_broadcast(128))
                    xbe[(e0, e1)] = t

            emit_bcast("s")
            emit_bcast("g")

            # ---------- q stats, pipelined by 512-column halves ----------
            pb, sq, diff, qnb, dvk = [], [], [], [], []
            for md in range(MD):
                pbt = bp16.tile([128, N], BF16, tag=f"pb{md}")
                pb.append(pbt)
                sqt = bp16.tile([128, N], BF16, tag=f"sq{md}")
                sq.append(sqt)
            mean_ps = ps.tile([128, N], F32, tag="ps")
            msq_ps = ps.tile([128, N], F32, tag="ps")
            mb2 = wtile()
            var = wtile()
            sd = wtile()
            for md in range(MD):
                dft = wtile()
                diff.append(dft)
            rsd = keep.tile([128, N], F32, tag="rsd")
            for md in range(MD):
                qnt = keep.tile([128, N], BF16, tag=f"qn{md}")
                qnb.append(qnt)
            qnb_pool = keep.tile([128, N], BF16, tag="qnp")
            if not simple:
                for md in range(MD):
                    dvt = keep.tile([128, N], F32, tag=f"dv{md}")
                    dvk.append(dvt)

            # ---------- product helpers ----------
            def xbe_of(e0, e1):
                for (b0, b1), t in xbe.items():
                    if b0 <= e0 and e1 <= b1:
                        return t, b0
                raise AssertionError((e0, e1))

            out_view = []
            for md in range(MD):
                ov = out_d[4096 * md : 4096 * (md + 1), :].rearrange(
                    "(p e) n -> p e n", e=32
                )
                out_view.append(ov)

            otile = {}
            for md, j in _ALLOC_ORDER:
                ot = op.tile([128, 4 * N], BF16, tag="ot")
                otile[(md, j)] = ot

            def emit_mul(e_, qsrc, md, e0, e1, h=None):
                j = e0 // 4
                assert e1 <= 4 * (j + 1)
                o = otile[(md, j)]
                xt, b0 = xbe_of(e0, e1)
                e_.tensor_tensor(
                    sub_ap(o, e0 - 4 * j, e1 - 4 * j, h)
                    if h is not None
                    else o[:, (e0 - 4 * j) * N : (e1 - 4 * j) * N],
                    rep_ap(qsrc, e1 - e0, h),
                    sub_ap(xt, e0 - b0, e1 - b0, h),
                    op=MULT)

            def emit_out(md, j):
                o = otile[(md, j)]
                eng[_OUT_ENG[md][j]].dma_start(
                    out_view[md][:, 4 * j : 4 * (j + 1), :], o[:])

            # DVE mul order within a half: by e (broadcast arrival order)
            vseq = sorted(
                [(0, c) for c in _MUL_V[0]] + [(1, c) for c in _MUL_V[1]],
                key=lambda mc: (mc[1][0], mc[0]))

            # ---------- stats chain + muls, pipelined by halves ----------
            def emit_pb_sq(hs):
                for md in range(MD):
                    if simple:
                        nc.scalar.copy(pb[md][:, hs], proj[md][:, hs])
                    else:
                        nc.vector.tensor_scalar(pb[md][:, hs],
                                                proj[md][:, hs],
                                                bp_sb[:, md : md + 1], None,
                                                op0=ADD)
                    e_ = nc.vector if md == 0 else nc.gpsimd
                    e_.tensor_tensor(sq[md][:, hs], pb[md][:, hs],
                                     pb[md][:, hs], op=MULT)

            def emit_mms(hs):
                for md in range(MD):
                    nc.tensor.matmul(mean_ps[:, hs], onesq[:], pb[md][:, hs],
                                     start=(md == 0), stop=(md == MD - 1))
                for md in range(MD):
                    nc.tensor.matmul(msq_ps[:, hs], onesq[:], sq[md][:, hs],
                                     start=(md == 0), stop=(md == MD - 1))

            def emit_chain(hs):
                nc.scalar.square(mb2[:, hs], mean_ps[:, hs])
                nc.vector.tensor_tensor(diff[0][:, hs], pb[0][:, hs],
                                        mean_ps[:, hs], op=SUB)
                nc.vector.tensor_tensor(var[:, hs], msq_ps[:, hs],
                                        mb2[:, hs], op=SUB)
                nc.scalar.activation(sd[:, hs], var[:, hs], SQRT,
                                     bias=eps_t[:])
                nc.vector.tensor_tensor(diff[1][:, hs], pb[1][:, hs],
                                        mean_ps[:, hs], op=SUB)
                nc.vector.reciprocal_approx_fast(rsd[:, hs], sd[:, hs])
                if simple:
                    nc.vector.tensor_tensor(qnb[0][:, hs], diff[0][:, hs],
                                            rsd[:, hs], op=MULT)
                    # qn[1] lives only in Pool's copy; DVE md1 muls read it
                    nc.gpsimd.tensor_tensor(qnb_pool[:, hs], diff[1][:, hs],
                                            rsd[:, hs], op=MULT)
                else:
                    for md in range(MD):
                        nc.vector.tensor_tensor(dvk[md][:, hs],
                                                diff[md][:, hs],
                                                rsd[:, hs], op=MULT)
                        nc.vector.tensor_scalar(qnb[md][:, hs],
                                                dvk[md][:, hs],
                                                g1_sb[:, md : md + 1],
                                                b1_sb[:, md : md + 1],
                                                op0=MULT, op1=ADD)
                    nc.gpsimd.tensor_copy(qnb_pool[:, hs], qnb[1][:, hs])

            def emit_muls(hi):
                q1src = qnb_pool if simple else qnb[1]
                with tc.high_priority():
                    for e0, e1 in _MUL_G[1]:
                        emit_mul(nc.gpsimd, qnb_pool, 1, e0, e1, hi)
                    for md, (e0, e1) in vseq:
                        emit_mul(nc.vector,
                                 qnb[0] if md == 0 else q1src, md, e0, e1, hi)

            h0, h1 = halves
            emit_pb_sq(h0)
            emit_mms(h0)
            emit_chain(h0)
            # Act h1 stats ops are emitted before the Act broadcasts so the
            # bcast DMAs don't block the h1 chain on the Act queue.
            emit_pb_sq(h1)
            emit_mms(h1)
            emit_bcast("a")
            emit_muls(0)
            emit_chain(h1)
            emit_muls(1)

            # output DMAs
            for md, j in _ALLOC_ORDER:
                emit_out(md, j)

    nc.compile()
    return nc


def _host_inputs(q, x, Wp, bp, g1, b1, g2, b2):
    """Build the 8 per-core input maps."""
    import os

    import ml_dtypes

    simple = os.environ.get("HM_SIMPLE", "0") == "1"
    qf = np.asarray(q, dtype=np.float32).reshape(B, C1, N)
    qb = np.ascontiguousarray(qf).astype(ml_dtypes.bfloat16)
    xf = np.ascontiguousarray(
        np.asarray(x, dtype=np.float32).reshape(B, C2, N)
    ).astype(ml_dtypes.bfloat16)
    wpt = np.ascontiguousarray(np.asarray(Wp, dtype=np.float32).T).astype(
        ml_dtypes.bfloat16
    )
    in_maps = []
    for b in range(B):
        m = {
            "q": np.ascontiguousarray(qb[b]),
            "w": wpt,
            "x": np.ascontiguousarray(xf[b]),
        }
        if not simple:
            m["bpc"] = np.ascontiguousarray(
                np.asarray(bp, dtype=np.float32).reshape(MD, 128).T)
            m["g1c"] = np.ascontiguousarray(
                np.asarray(g1, dtype=np.float32).reshape(MD, 128).T)
            m["b1c"] = np.ascontiguousarray(
                np.asarray(b1, dtype=np.float32).reshape(MD, 128).T)
            m["g2r"] = np.ascontiguousarray(
                np.asarray(g2, dtype=np.float32)[:, None])
            m["b2r"] = np.ascontiguousarray(
                np.asarray(b2, dtype=np.float32)[:, None])
        in_maps.append(m)
    return in_maps


def _run(in_maps, trace=False):
    import os

    from concourse.bass_utils import run_bass_kernel_spmd

    key = "nc" + os.environ.get("HM_SIMPLE", "0")
    if key not in _CACHE:
        _CACHE[key] = _build_nc(os.environ.get("HM_SIMPLE", "0") == "1")
    nc = _CACHE[key]
    res = run_bass_kernel_spmd(nc, in_maps, core_ids=list(range(B)), trace=trace)
    return res


def kernel(q, x, Wp, bp, g1, b1, g2, b2):
    import os

    simple = (
        np.allclose(np.asarray(bp), 0)
        and np.allclose(np.asarray(g1), 1)
        and np.allclose(np.asarray(b1), 0)
        and np.allclose(np.asarray(g2), 1)
        and np.allclose(np.asarray(b2), 0)
    )
    os.environ["HM_SIMPLE"] = "1" if simple else "0"
    in_maps = _host_inputs(q, x, Wp, bp, g1, b1, g2, b2)
    res = _run(in_maps, trace=False)
    out = np.stack(
        [
            np.asarray(res.results[b]["out"]).astype(np.float32).reshape(CD, H, W)
            for b in range(B)
        ]
    )
    _CACHE["last_res"] = res
    return out


# revision 32
# speedup vs baseline: 2.2195x; 1.0388x over previous
"""Trainium2 Bass kernel for nn_HadaMard: fused proj + 2xLayerNorm + outer product.

Reference computation (per batch b):
  qf = q[b].reshape(C1, N)           # [1024, 1024]  (C1 rows, N=H*W cols)
  proj = Wp @ qf + bp                # [256, 1024]
  qn = LN_over_d(proj) * g1 + b1     # LN over the 256-channel dim
  xn = LN_over_e(x[b]) * g2 + b2     # LN over the 32-channel dim
  out[d*32+e, n] = qn[d, n] * xn[e, n]   # [8192, 1024]

Sharding: data-parallel over B=8, one batch per NeuronCore.

Layout ("flipped tiling"): output tiles keep qn's channel dim d on the
partitions (dblock in {0,1} x 128 partitions) and iterate e in the free dim.
  - proj: PE matmuls (bf16), accumulated in f32 PSUM, k-loop ordered by
    DMA arrival; the q-stats/LN chain is pipelined by 512-column halves
    so qn's first half is ready early.
  - LN stats via bf16 ones-matmuls; 1/sd via reciprocal_approx_fast.
  - xn (32 rows, bf16) replicated to 128 partitions via DRAM-roundtrip
    DMAs with stride-0 source (partition_broadcast); one scratch copy per
    issuing engine keeps the read ordered behind the write in-queue.
  - product: all-bf16 tensor_tensor multiplies (DVE 2x mode) with the qn
    operand repeated along the free dim via a stride-0 AP; ~1/3 of the
    chunks run on the Pool engine.
  - output: bf16 DRAM tensor (host converts to f32), 4-e-wide tiles,
    DMAs spread across SP / Act / Pool.

Axon-backend constraints honored: no float32r matmuls, no AluOp.divide,
at most one PSUM operand per DVE op, no PSUM operands on Pool, DMA only
on SP / Act / Pool.
"""

import numpy as np

_CACHE = {}

B, C1, H, W = 8, 1024, 32, 32
C2 = 32
Cp = 256
N = 1024
CD = Cp * C2  # 8192
MD = Cp // 128  # 2 row-blocks of proj/qn
EPS = 1e-5

# mul chunks (e0, e1) per dblock for DVE ('v') and Pool ('g').
# DVE chunks are emitted per column-half; Pool chunks are full-width.
_MUL_V = {
    0: [(0, 2), (2, 4), (4, 8), (8, 12), (12, 16), (16, 20), (20, 24),
        (24, 28), (28, 32)],
    1: [(12, 16), (16, 20), (20, 24)],
}
_MUL_G = {
    0: [],
    1: [(0, 2), (2, 4), (4, 6), (6, 8), (8, 10), (10, 12), (24, 26), (26, 28),
        (28, 30), (30, 32)],
}
# xn broadcast chunks: (e0, e1, engine): 's' SP, 'a' Act, 'g' Pool
_BCAST = [(0, 2, "s"), (2, 4, "s"), (4, 8, "s"), (8, 12, "s"), (12, 16, "s"),
          (16, 20, "a"), (20, 24, "a"), (24, 32, "g")]
# output tiles per dblock: 8 x 4-e tiles, (j -> dma engine)
_OUT_ENG = {
    0: ["s", "a", "s", "a", "s", "a", "s", "a"],
    1: ["a", "g", "s", "a", "g", "a", "g", "s"],
}
# O-tile allocation order (rough completion order; pool bufs=6)
_ALLOC_ORDER = [(0, 0), (0, 1), (1, 0), (0, 2), (1, 1), (0, 3), (1, 2),
                (1, 3), (0, 4), (1, 4), (0, 5), (1, 5), (0, 6), (1, 6),
                (0, 7), (1, 7)]
# q-tile k -> load engine; proj accumulation follows arrival order
_Q_ENG = {0: "s", 3: "s", 1: "a", 4: "a", 2: "g", 5: "g", 6: "g", 7: "g"}
_K_ORDER = [2, 0, 1, 5, 3, 4, 6, 7]


def _build_nc(simple):
    import concourse.bacc as bacc
    import concourse.bass as bass
    import concourse.mybir as mybir
    import concourse.tile as tile

    F32 = mybir.dt.float32
    BF16 = mybir.dt.bfloat16
    MULT = mybir.AluOpType.mult
    SUB = mybir.AluOpType.subtract
    ADD = mybir.AluOpType.add
    SQRT = mybir.ActivationFunctionType.Sqrt

    nc = bacc.Bacc(None, target_bir_lowering=False)

    q_d = nc.dram_tensor("q", [C1, N], BF16, kind="ExternalInput")
    w_d = nc.dram_tensor("w", [C1, Cp], BF16, kind="ExternalInput")
    x_d = nc.dram_tensor("x", [C2, N], BF16, kind="ExternalInput")
    if not simple:
        bp_d = nc.dram_tensor("bpc", [128, MD], F32, kind="ExternalInput")
        g1_d = nc.dram_tensor("g1c", [128, MD], F32, kind="ExternalInput")
        b1_d = nc.dram_tensor("b1c", [128, MD], F32, kind="ExternalInput")
        g2_d = nc.dram_tensor("g2r", [C2, 1], F32, kind="ExternalInput")
        b2_d = nc.dram_tensor("b2r", [C2, 1], F32, kind="ExternalInput")
    xs_d = {
        "s": nc.dram_tensor("xs0", [C2, N], BF16, kind="ExternalOutput"),
        "a": nc.dram_tensor("xs1", [C2, N], BF16, kind="ExternalOutput"),
        "g": nc.dram_tensor("xs2", [C2, N], BF16, kind="ExternalOutput"),
    }
    out_d = nc.dram_tensor("out", [CD, N], BF16, kind="ExternalOutput")

    def rep_ap(t, r, h=None):
        """qn tile AP repeated r times along a stride-0 free dim.

        h=None: full rows; h=0/1: 512-column half (offset 512h)."""
        a = t[:].copy()
        while len(a.ap) > 0:
            a.ap.pop()
        a.ap.append([N, 128])
        a.ap.append([0, r])
        if h is None:
            a.ap.append([1, N])
        else:
            a.ap.append([1, 512])
            a.offset = a.offset + 512 * h
        return a

    def sub_ap(t, e0, e1, h=None):
        """e-major tile viewed as (p, e, n): slice e and optionally a
        512-col half of n.  Partition stride taken from the tile itself."""
        a = t[:].copy()
        base = a.offset
        pstride = a.ap[0][0]
        while len(a.ap) > 0:
            a.ap.pop()
        a.ap.append([pstride, 128])
        a.ap.append([N, e1 - e0])
        if h is None:
            a.ap.append([1, N])
            a.offset = base + e0 * N
        else:
            a.ap.append([1, 512])
            a.offset = base + e0 * N + 512 * h
        return a

    with tile.TileContext(nc) as tc:
        with (
            tc.tile_pool(name="cst", bufs=1) as cst,
            tc.tile_pool(name="qp", bufs=1) as qp,
            tc.tile_pool(name="wrk", bufs=1) as wrk,
            tc.tile_pool(name="bp16", bufs=1) as bp16,
            tc.tile_pool(name="keep", bufs=1) as keep,
            tc.tile_pool(name="xbe", bufs=1) as xbep,
            tc.tile_pool(name="op", bufs=9) as op,
            tc.tile_pool(name="ps", bufs=4, space=bass.MemorySpace.PSUM) as ps,
        ):
            eng = {"s": nc.sync, "a": nc.scalar, "g": nc.gpsimd}

            # ---------- constants / memsets (DVE) ----------
            onesx = cst.tile([C2, C2], BF16, tag="onesx")
            nc.vector.memset(onesx[:], 1.0 / C2)
            onesq = cst.tile([128, 128], BF16, tag="onesq")
            nc.vector.memset(onesq[:], 1.0 / Cp)
            eps_t = cst.tile([128, 1], F32, tag="eps")
            nc.vector.memset(eps_t[:], EPS)

            _wn = [0]

            def wtile():
                t = wrk.tile([128, N], F32, tag=f"t{_wn[0] % 5}")
                _wn[0] += 1
                return t

            halves = [slice(0, 512), slice(512, 1024)]

            # ---------- input loads ----------
            # SP: w0, q0, x, q3; Act: w1, q1, q4; Pool: q2, q5, q6, q7
            wg = []
            for g in range(2):
                t = cst.tile([128, 4 * Cp], BF16, tag=f"w{g}")
                dst = t[:].rearrange("p (k d) -> p k d", k=4)
                src = w_d[512 * g : 512 * (g + 1), :].rearrange(
                    "(k p) d -> p k d", k=4
                )
                [nc.sync, nc.scalar][g].dma_start(dst, src)
                wg.append(t)
            xsb = cst.tile([C2, N], BF16, tag="x")
            nc.sync.dma_start(xsb[:], x_d[:])
            q_sb = {}
            for k in [2, 0, 1, 5, 3, 4, 6, 7]:
                t = qp.tile([128, N], BF16, tag=f"q{k}")
                eng[_Q_ENG[k]].dma_start(t[:], q_d[128 * k : 128 * (k + 1), :])
                q_sb[k] = t
            if not simple:
                bp_sb = cst.tile([128, MD], F32, tag="bp")
                nc.sync.dma_start(bp_sb[:], bp_d[:])
                g1_sb = cst.tile([128, MD], F32, tag="g1")
                nc.sync.dma_start(g1_sb[:], g1_d[:])
                b1_sb = cst.tile([128, MD], F32, tag="b1")
                nc.scalar.dma_start(b1_sb[:], b1_d[:])
                g2_sb = cst.tile([C2, 1], F32, tag="g2")
                nc.scalar.dma_start(g2_sb[:], g2_d[:])
                b2_sb = cst.tile([C2, 1], F32, tag="b2")
                nc.scalar.dma_start(b2_sb[:], b2_d[:])

            # ---------- x stats + first part of proj (PE) ----------
            xsq = bp16.tile([C2, N], BF16, tag="xsq")
            nc.vector.tensor_tensor(xsq[:], xsb[:], xsb[:], op=MULT)
            mx_ps = ps.tile([128, N], F32, tag="ps")
            mxq_ps = ps.tile([128, N], F32, tag="ps")

            proj = []
            for md in range(MD):
                pj = ps.tile([128, N], F32, tag="ps")
                proj.append(pj)

            def proj_mms(i, k):
                for hs in halves:
                    for md in range(MD):
                        lh = wg[k // 4][:, (k % 4) * Cp + 128 * md :
                                        (k % 4) * Cp + 128 * (md + 1)]
                        nc.tensor.matmul(proj[md][:, hs], lh, q_sb[k][:, hs],
                                         start=(i == 0), stop=(i == 7))

            # PE queue: proj[k2, k0], x-stat mms, proj[rest]
            proj_mms(0, _K_ORDER[0])
            proj_mms(1, _K_ORDER[1])
            for hs in halves:
                nc.tensor.matmul(mx_ps[:C2, hs], onesx[:], xsb[:, hs],
                                 start=True, stop=True)
            for hs in halves:
                nc.tensor.matmul(mxq_ps[:C2, hs], onesx[:], xsq[:, hs],
                                 start=True, stop=True)
            for i in range(2, 8):
                proj_mms(i, _K_ORDER[i])

            # ---------- x LN (half-pipelined chain) ----------
            mx2 = wtile()
            xd = wtile()
            varx = wtile()
            sdx = wtile()
            rsdx = keep.tile([C2, N], F32, tag="rsdx")
            xn = keep.tile([C2, N], BF16, tag="xn")
            if not simple:
                xtmp = wtile()
            for hs in halves:
                nc.scalar.square(mx2[:C2, hs], mx_ps[:C2, hs])
                nc.vector.tensor_tensor(xd[:C2, hs], xsb[:, hs],
                                        mx_ps[:C2, hs], op=SUB)
                nc.vector.tensor_tensor(varx[:C2, hs], mxq_ps[:C2, hs],
                                        mx2[:C2, hs], op=SUB)
                nc.scalar.activation(sdx[:C2, hs], varx[:C2, hs], SQRT,
                                     bias=eps_t[:C2, :])
                nc.vector.reciprocal_approx_fast(rsdx[:, hs], sdx[:C2, hs])
                if simple:
                    nc.vector.tensor_tensor(xn[:, hs], xd[:C2, hs],
                                            rsdx[:, hs], op=MULT)
                else:
                    nc.vector.tensor_tensor(xtmp[:C2, hs], xd[:C2, hs],
                                            rsdx[:, hs], op=MULT)
                    nc.vector.tensor_scalar(xn[:, hs], xtmp[:C2, hs],
                                            g2_sb[:], b2_sb[:],
                                            op0=MULT, op1=ADD)

            # ---------- xn scratch writes + stride-0 broadcasts ----------
            xbe = {}
            _bc_written = set()

            def emit_bcast(which):
                for e0, e1, en in _BCAST:
                    if en != which:
                        continue
                    if which not in _bc_written:
                        eng[which].dma_start(xs_d[which][:], xn[:])
                        _bc_written.add(which)
                    t = xbep.tile([128, (e1 - e0) * N], BF16,
                                  tag=f"xbe{e0}")
                    eng[which].dma_start(
                        t[:], xs_d[which][e0:e1, :].partition_broadcast(128))
                    xbe[(e0, e1)] = t

            emit_bcast("s")
            emit_bcast("g")

            # ---------- q stats, pipelined by 512-column halves ----------
            pb, sq, diff, qnb, dvk = [], [], [], [], []
            for md in range(MD):
                pbt = bp16.tile([128, N], BF16, tag=f"pb{md}")
                pb.append(pbt)
                sqt = bp16.tile([128, N], BF16, tag=f"sq{md}")
                sq.append(sqt)
            mean_ps = ps.tile([128, N], F32, tag="ps")
            msq_ps = ps.tile([128, N], F32, tag="ps")
            mb2 = wtile()
            var = wtile()
            sd = wtile()
            for md in range(MD):
                dft = wtile()
                diff.append(dft)
            rsd = keep.tile([128, N], F32, tag="rsd")
            for md in range(MD):
                qnt = keep.tile([128, N], BF16, tag=f"qn{md}")
                qnb.append(qnt)
            qnb_pool = keep.tile([128, N], BF16, tag="qnp")
            if not simple:
                for md in range(MD):
                    dvt = keep.tile([128, N], F32, tag=f"dv{md}")
                    dvk.append(dvt)

            # ---------- product helpers ----------
            def xbe_of(e0, e1):
                for (b0, b1), t in xbe.items():
                    if b0 <= e0 and e1 <= b1:
                        return t, b0
                raise AssertionError((e0, e1))

            out_view = []
            for md in range(MD):
                ov = out_d[4096 * md : 4096 * (md + 1), :].rearrange(
                    "(p e) n -> p e n", e=32
                )
                out_view.append(ov)

            otile = {}
            for md, j in _ALLOC_ORDER:
                ot = op.tile([128, 4 * N], BF16, tag="ot")
                otile[(md, j)] = ot

            def emit_mul(e_, qsrc, md, e0, e1, h=None):
                j = e0 // 4
                assert e1 <= 4 * (j + 1)
                o = otile[(md, j)]
                xt, b0 = xbe_of(e0, e1)
                e_.tensor_tensor(
                    sub_ap(o, e0 - 4 * j, e1 - 4 * j, h)
                    if h is not None
                    else o[:, (e0 - 4 * j) * N : (e1 - 4 * j) * N],
                    rep_ap(qsrc, e1 - e0, h),
                    sub_ap(xt, e0 - b0, e1 - b0, h),
                    op=MULT)

            def emit_out(md, j):
                o = otile[(md, j)]
                eng[_OUT_ENG[md][j]].dma_start(
                    out_view[md][:, 4 * j : 4 * (j + 1), :], o[:])

            # DVE mul order within a half: by e (broadcast arrival order)
            vseq = sorted(
                [(0, c) for c in _MUL_V[0]] + [(1, c) for c in _MUL_V[1]],
                key=lambda mc: (mc[1][0], mc[0]))

            # ---------- stats chain + muls, pipelined by halves ----------
            def emit_pb_sq(hs):
                for md in range(MD):
                    if simple:
                        nc.scalar.copy(pb[md][:, hs], proj[md][:, hs])
                    else:
                        nc.vector.tensor_scalar(pb[md][:, hs],
                                                proj[md][:, hs],
                                                bp_sb[:, md : md + 1], None,
                                                op0=ADD)
                    e_ = nc.vector if md == 0 else nc.gpsimd
                    e_.tensor_tensor(sq[md][:, hs], pb[md][:, hs],
                                     pb[md][:, hs], op=MULT)

            def emit_mms(hs):
                for md in range(MD):
                    nc.tensor.matmul(mean_ps[:, hs], onesq[:], pb[md][:, hs],
                                     start=(md == 0), stop=(md == MD - 1))
                for md in range(MD):
                    nc.tensor.matmul(msq_ps[:, hs], onesq[:], sq[md][:, hs],
                                     start=(md == 0), stop=(md == MD - 1))

            def emit_chain(hs):
                nc.scalar.square(mb2[:, hs], mean_ps[:, hs])
                nc.vector.tensor_tensor(diff[0][:, hs], pb[0][:, hs],
                                        mean_ps[:, hs], op=SUB)
                nc.vector.tensor_tensor(var[:, hs], msq_ps[:, hs],
                                        mb2[:, hs], op=SUB)
                nc.scalar.activation(sd[:, hs], var[:, hs], SQRT,
                                     bias=eps_t[:])
                nc.vector.tensor_tensor(diff[1][:, hs], pb[1][:, hs],
                                        mean_ps[:, hs], op=SUB)
                nc.vector.reciprocal_approx_fast(rsd[:, hs], sd[:, hs])
                if simple:
                    nc.vector.tensor_tensor(qnb[0][:, hs], diff[0][:, hs],
                                            rsd[:, hs], op=MULT)
                    # qn[1] lives only in Pool's copy; DVE md1 muls read it
                    nc.gpsimd.tensor_tensor(qnb_pool[:, hs], diff[1][:, hs],
                                            rsd[:, hs], op=MULT)
                else:
                    for md in range(MD):
                        nc.vector.tensor_tensor(dvk[md][:, hs],
                                                diff[md][:, hs],
                                                rsd[:, hs], op=MULT)
                        nc.vector.tensor_scalar(qnb[md][:, hs],
                                                dvk[md][:, hs],
                                                g1_sb[:, md : md + 1],
                                                b1_sb[:, md : md + 1],
                                                op0=MULT, op1=ADD)
                    nc.gpsimd.tensor_copy(qnb_pool[:, hs], qnb[1][:, hs])

            def emit_muls(hi):
                q1src = qnb_pool if simple else qnb[1]
                with tc.high_priority():
                    for e0, e1 in _MUL_G[1]:
                        emit_mul(nc.gpsimd, qnb_pool, 1, e0, e1, hi)
                    for md, (e0, e1) in vseq:
                        emit_mul(nc.vector,
                                 qnb[0] if md == 0 else q1src, md, e0, e1, hi)

            h0, h1 = halves
            emit_pb_sq(h0)
            emit_mms(h0)
            emit_chain(h0)
            # Act h1 stats ops are emitted before the Act broadcasts so the
            # bcast DMAs don't block the h1 chain on the Act queue.
            emit_pb_sq(h1)
            emit_mms(h1)
            emit_bcast("a")
            emit_muls(0)
            emit_chain(h1)
            emit_muls(1)

            # output DMAs
            for md, j in _ALLOC_ORDER:
                emit_out(md, j)

    nc.compile()
    return nc


def _host_inputs(q, x, Wp, bp, g1, b1, g2, b2):
    """Build the 8 per-core input maps."""
    import os

    import ml_dtypes

    simple = os.environ.get("HM_SIMPLE", "0") == "1"
    qf = np.asarray(q, dtype=np.float32).reshape(B, C1, N)
    qb = np.ascontiguousarray(qf).astype(ml_dtypes.bfloat16)
    xf = np.ascontiguousarray(
        np.asarray(x, dtype=np.float32).reshape(B, C2, N)
    ).astype(ml_dtypes.bfloat16)
    wpt = np.ascontiguousarray(np.asarray(Wp, dtype=np.float32).T).astype(
        ml_dtypes.bfloat16
    )
    in_maps = []
    for b in range(B):
        m = {
            "q": np.ascontiguousarray(qb[b]),
            "w": wpt,
            "x": np.ascontiguousarray(xf[b]),
        }
        if not simple:
            m["bpc"] = np.ascontiguousarray(
                np.asarray(bp, dtype=np.float32).reshape(MD, 128).T)
            m["g1c"] = np.ascontiguousarray(
                np.asarray(g1, dtype=np.float32).reshape(MD, 128).T)
            m["b1c"] = np.ascontiguousarray(
                np.asarray(b1, dtype=np.float32).reshape(MD, 128).T)
            m["g2r"] = np.ascontiguousarray(
                np.asarray(g2, dtype=np.float32)[:, None])
            m["b2r"] = np.ascontiguousarray(
                np.asarray(b2, dtype=np.float32)[:, None])
        in_maps.append(m)
    return in_maps


def _run(in_maps, trace=False):
    import os

    from concourse.bass_utils import run_bass_kernel_spmd

    key = "nc" + os.environ.get("HM_SIMPLE", "0")
    if key not in _CACHE:
        _CACHE[key] = _build_nc(os.environ.get("HM_SIMPLE", "0") == "1")
    nc = _CACHE[key]
    res = run_bass_kernel_spmd(nc, in_maps, core_ids=list(range(B)), trace=trace)
    return res


def kernel(q, x, Wp, bp, g1, b1, g2, b2):
    import os

    simple = (
        np.allclose(np.asarray(bp), 0)
        and np.allclose(np.asarray(g1), 1)
        and np.allclose(np.asarray(b1), 0)
        and np.allclose(np.asarray(g2), 1)
        and np.allclose(np.asarray(b2), 0)
    )
    os.environ["HM_SIMPLE"] = "1" if simple else "0"
    in_maps = _host_inputs(q, x, Wp, bp, g1, b1, g2, b2)
    res = _run(in_maps, trace=False)
    out = np.stack(
        [
            np.asarray(res.results[b]["out"]).astype(np.float32).reshape(CD, H, W)
            for b in range(B)
        ]
    )
    _CACHE["last_res"] = res
    return out
